# revision 13
# baseline (speedup 1.0000x reference)
"""Edge-parallel GNN message-passing layer on 8 TRN2 NeuronCores.

Sharding: each core owns NQ/8 query nodes and all edges pointing at them
(host pre-sorts edges by destination block); node features/weights are
replicated, so no collectives are needed — each core produces its output
rows independently.

Math notes: the reference's segment-max subtraction cancels exactly in
msg/denom and scores are small (|e| < ~5), so exp() is computed directly;
1/x and 1/sqrt(x) are computed as exp(-ln(x)) / exp(-0.5 ln(x)) so the
whole kernel stays in one ACT table set (natural_log_exp_and_others).
"""

import numpy as np
import ml_dtypes

BF16 = ml_dtypes.bfloat16

N_CORES = 8
DIM = 128
H = 8
DH = 16
CHUNK = 32768  # dma_gather int16 index reach (rows per gather table window)
ST = 4   # tiles per super-tile (512 edge slots)
GMAX = 8  # tiles per gather call (SWDGE descriptor-ring limit ~1024)
LN_EPS = 1e-5

USE_ACT_PRELU = True  # HW table has Prelu; interp does not — disable for sim

_CACHE = {}


def _subcalls(tcc):
    """Gather sub-call tile counts for a (block, chunk) with tcc tiles."""
    return [min(GMAX, tcc - g0) for g0 in range(0, tcc, GMAX)]


# ----------------------------------------------------------------------------
# Host-side prep
# ----------------------------------------------------------------------------


def _prep(query_idx, key_idx, nq, nk):
    npc = nq // N_CORES
    nblk = (npc + 127) // 128
    nch = (nk + CHUNK - 1) // CHUNK

    qi = np.asarray(query_idx).astype(np.int64)
    ki = np.asarray(key_idx).astype(np.int64)
    core = qi // npc

    per_core = []
    counts_all = np.zeros((N_CORES, nblk, nch), np.int64)
    for c in range(N_CORES):
        sel = np.nonzero(core == c)[0]
        eq = qi[sel] - c * npc
        ek = ki[sel]
        grp = (eq >> 7) * nch + (ek >> 15)
        order = np.argsort(grp, kind="stable")
        per_core.append((grp[order], (eq & 127)[order], (ek & (CHUNK - 1))[order]))
        counts_all[c] = np.bincount(grp, minlength=nblk * nch).reshape(nblk, nch)

    # Static tile schedule: capacity must cover max-core count plus one
    # dummy slot per gather sub-call (guarantees every call has >=1 valid idx).
    maxc = counts_all.max(axis=0)  # [nblk, nch]
    T = np.zeros((nblk, nch), np.int64)
    for b in range(nblk):
        for ch in range(nch):
            t = max(1, (int(maxc[b, ch]) + 127) // 128)
            while int(maxc[b, ch]) + len(_subcalls(t)) > t * 128:
                t += 1
            T[b, ch] = t
    tot = T.sum(axis=1)
    T[:, nch - 1] += (-tot) % ST
    flat = T.reshape(-1)
    tb = np.concatenate([[0], np.cumsum(flat)[:-1]]).reshape(nblk, nch)
    total_tiles = int(flat.sum())
    S = total_tiles * 128

    # Per-(b,ch): one dummy slot at each gather sub-call start; the rest is
    # capacity for real edges (packed ascending), then trailing -1 padding.
    ncalls = 0
    dummy_slots = []
    call_meta = []
    for b in range(nblk):
        for ch in range(nch):
            tcc = int(T[b, ch])
            base_slot = int(tb[b, ch]) * 128
            for g0 in range(0, tcc, GMAX):
                dummy_slots.append(base_slot + g0 * 128)
                call_meta.append((b, ch, g0, min(GMAX, tcc - g0)))
                ncalls += 1
    dummy_slots = np.array(dummy_slots, np.int64)
    is_dummy = np.zeros(S, bool)
    is_dummy[dummy_slots] = True
    cap_pos_all = np.nonzero(~is_dummy)[0]

    grp_start_slot = tb.reshape(-1) * 128

    kvidx = np.full((N_CORES, S), -1, np.int16)
    kvidx[:, dummy_slots] = 0
    qrel = np.full((N_CORES, S), -1.0, np.float32)
    gcnt = np.zeros((N_CORES, ncalls), np.int32)

    call_of_slot = np.zeros(S, np.int32)
    for ci, (b, ch, g0, gt) in enumerate(call_meta):
        s0 = (int(tb[b, ch]) + g0) * 128
        call_of_slot[s0:s0 + gt * 128] = ci
    dummy_calls = np.bincount(call_of_slot[dummy_slots], minlength=ncalls).astype(np.int32)

    for c in range(N_CORES):
        grp_s, qrel_s, loc_s = per_core[c]
        cnt = counts_all[c].reshape(-1)
        starts = np.concatenate([[0], np.cumsum(cnt)[:-1]])
        rank = np.arange(grp_s.shape[0]) - np.repeat(starts, cnt)
        lo = np.searchsorted(cap_pos_all, grp_start_slot[grp_s])
        slot = cap_pos_all[lo + rank]
        kvidx[c, slot] = loc_s.astype(np.int16)
        qrel[c, slot] = qrel_s.astype(np.float32)
        gcnt[c] = np.bincount(call_of_slot[slot], minlength=ncalls).astype(np.int32)
        gcnt[c] += dummy_calls

    kvidx_w = np.ascontiguousarray(kvidx.reshape(N_CORES, -1, 16).transpose(0, 2, 1))
    kvidx_w = np.ascontiguousarray(np.tile(kvidx_w, (1, 8, 1)))
    qrelp = np.ascontiguousarray(
        qrel.reshape(N_CORES, -1, 128).transpose(0, 2, 1)).astype(BF16)
    qrelr = qrel.reshape(N_CORES, 1, S).astype(BF16)

    return {
        "npc": npc, "nblk": nblk, "nch": nch, "T": T, "tile_base": tb,
        "total_tiles": total_tiles, "S": S, "ncalls": ncalls,
        "T_MAX": int(T.sum(axis=1).max()),
        "kvidx": kvidx_w, "qrelp": qrelp, "qrelr": qrelr, "gcnt": gcnt,
    }


# ----------------------------------------------------------------------------
# Device kernel
# ----------------------------------------------------------------------------


def _build(sched, nq, nk, has_bias):
    import concourse.bacc as bacc
    import concourse.mybir as mybir
    import concourse.tile as tile

    dt = mybir.dt
    Alu = mybir.AluOpType
    Act = mybir.ActivationFunctionType

    nblk, nch = sched["nblk"], sched["nch"]
    T, tile_base = sched["T"], sched["tile_base"]
    S, ncalls, T_MAX = sched["S"], sched["ncalls"], sched["T_MAX"]
    npc_pad = nblk * 128
    nk_pad = ((nk + 127) // 128) * 128
    nkc = nk_pad // 128

    nc = bacc.Bacc(None)

    p_kvidx = nc.declare_dram_parameter("kvidx", [128, S // 16], dt.int16, isOutput=False)
    p_qrelp = nc.declare_dram_parameter("qrelp", [128, S // 128], dt.bfloat16, isOutput=False)
    p_qrelr = nc.declare_dram_parameter("qrelr", [1, S], dt.bfloat16, isOutput=False)
    p_gcnt = nc.declare_dram_parameter("gcnt", [1, ncalls], dt.int32, isOutput=False)
    p_queryT = nc.declare_dram_parameter("queryT", [128, npc_pad], dt.bfloat16, isOutput=False)
    p_query = nc.declare_dram_parameter("query", [npc_pad, 128], dt.float32, isOutput=False)
    p_keysT = nc.declare_dram_parameter("keysT", [128, nk_pad], dt.bfloat16, isOutput=False)
    p_valuesT = nc.declare_dram_parameter("valuesT", [128, nk_pad], dt.bfloat16, isOutput=False)
    p_wq = nc.declare_dram_parameter("wq", [128, 128], dt.bfloat16, isOutput=False)
    p_wk = nc.declare_dram_parameter("wk", [128, 128], dt.bfloat16, isOutput=False)
    p_wv = nc.declare_dram_parameter("wv", [128, 128], dt.bfloat16, isOutput=False)
    p_wp = nc.declare_dram_parameter("wp", [128, 128], dt.bfloat16, isOutput=False)
    p_abc = nc.declare_dram_parameter("abc", [128, 128 * T_MAX], dt.bfloat16, isOutput=False)
    p_iotar = nc.declare_dram_parameter("iotar", [128, 128 * T_MAX], dt.bfloat16, isOutput=False)
    p_iotac = nc.declare_dram_parameter("iotac", [128, 1], dt.float32, isOutput=False)
    p_ident = nc.declare_dram_parameter("ident", [128, 128], dt.bfloat16, isOutput=False)
    p_ones = nc.declare_dram_parameter("ones", [1, 128], dt.bfloat16, isOutput=False)
    if has_bias:
        p_biases = nc.declare_dram_parameter("biases", [1, 512], dt.float32, isOutput=False)
        p_lngb = nc.declare_dram_parameter("lngb", [1, 256], dt.float32, isOutput=False)
    p_out = nc.declare_dram_parameter("out", [npc_pad, 128], dt.float32, isOutput=True)

    kv_dram = nc.dram_tensor("kv_table", [nk_pad, 256], dt.bfloat16)

    KVG = 8
    cnt_regs = [nc.gpsimd.alloc_register(f"gcnt_r{i}") for i in range(4)]

    with tile.TileContext(nc) as tc:
        with (
            tc.tile_pool(name="const", bufs=1) as cpool,
            tc.tile_pool(name="kstream", bufs=3) as kpool,
            tc.tile_pool(name="kvout", bufs=3) as kvopool,
            tc.tile_pool(name="blk", bufs=2) as bpool,
            tc.tile_pool(name="stile", bufs=3) as spool,
            tc.tile_pool(name="epi", bufs=2) as epool,
            tc.tile_pool(name="psA", bufs=2, space="PSUM") as psA,
            tc.tile_pool(name="psB", bufs=2, space="PSUM") as psB,
            tc.tile_pool(name="psC", bufs=2, space="PSUM") as psC,
            tc.tile_pool(name="psD", bufs=2, space="PSUM") as psD,
        ):
            def cload(param, shape, dtype):
                t = cpool.tile(shape, dtype, tag=param.name)
                nc.sync.dma_start(out=t[:], in_=param[:])
                return t

            eps30 = cpool.tile([128, 1], dt.float32, tag="eps30")
            nc.gpsimd.memset(eps30[:], 1e-30)
            eps5 = cpool.tile([128, 1], dt.float32, tag="eps5")
            nc.gpsimd.memset(eps5[:], LN_EPS)
            wq = cload(p_wq, [128, 128], dt.bfloat16)
            wk = cload(p_wk, [128, 128], dt.bfloat16)
            wv = cload(p_wv, [128, 128], dt.bfloat16)
            wp = cload(p_wp, [128, 128], dt.bfloat16)
            abc = cload(p_abc, [128, 128 * T_MAX], dt.bfloat16)
            iotar = cload(p_iotar, [128, 128 * T_MAX], dt.bfloat16)
            iotac = cload(p_iotac, [128, 1], dt.float32)
            ident = cload(p_ident, [128, 128], dt.bfloat16)
            ones = cload(p_ones, [1, 128], dt.bfloat16)
            kvidx = cload(p_kvidx, [128, S // 16], dt.int16)
            qrelp = cload(p_qrelp, [128, S // 128], dt.bfloat16)
            gcnt = cload(p_gcnt, [1, ncalls], dt.int32)
            queryT = cload(p_queryT, [128, npc_pad], dt.bfloat16)
            if has_bias:
                biases = cload(p_biases, [1, 512], dt.float32)
                lngb = cload(p_lngb, [1, 256], dt.float32)

            # ---- Phase A: KV table = [keys@Wk.T | values@Wv.T] bf16 ----
            for jg0 in range(0, nkc, KVG):
                gsz = min(KVG, nkc - jg0)
                kT = kpool.tile([128, 128 * KVG], dt.bfloat16, tag="kT")
                vT = kpool.tile([128, 128 * KVG], dt.bfloat16, tag="vT")
                nc.sync.dma_start(out=kT[:, 0:128 * gsz], in_=p_keysT[:, jg0 * 128:(jg0 + gsz) * 128])
                nc.sync.dma_start(out=vT[:, 0:128 * gsz], in_=p_valuesT[:, jg0 * 128:(jg0 + gsz) * 128])
                kv_sb = kvopool.tile([128, KVG, 256], dt.bfloat16, tag="kv_sb")
                for j in range(gsz):
                    ps = psA.tile([128, 512], dt.float32, tag="ps_kv")
                    if has_bias:
                        nc.tensor.matmul(ps[:, 0:256], lhsT=ones[:], rhs=biases[:, 0:256],
                                         start=True, stop=False)
                        nc.tensor.matmul(ps[:, 0:128], lhsT=kT[:, j * 128:(j + 1) * 128],
                                         rhs=wk[:], start=False, stop=False)
                        nc.tensor.matmul(ps[:, 128:256], lhsT=vT[:, j * 128:(j + 1) * 128],
                                         rhs=wv[:], start=False, stop=True)
                    else:
                        nc.tensor.matmul(ps[:, 0:128], lhsT=kT[:, j * 128:(j + 1) * 128],
                                         rhs=wk[:], start=True, stop=False)
                        nc.tensor.matmul(ps[:, 128:256], lhsT=vT[:, j * 128:(j + 1) * 128],
                                         rhs=wv[:], start=False, stop=True)
                    if j % 2 == 0:
                        nc.scalar.activation(kv_sb[:, j, :], ps[:, 0:256], Act.Copy)
                    else:
                        nc.vector.tensor_copy(kv_sb[:, j, :], ps[:, 0:256])
                dview = kv_dram[jg0 * 128:(jg0 + gsz) * 128, :]
                dview = dview.rearrange("(j p) d -> p j d", p=128)
                nc.sync.dma_start(out=dview, in_=kv_sb[:, 0:gsz, :])

            # ---- Phase B ----
            call_i = 0
            for b in range(nblk):
                tb0 = int(tile_base[b, 0])
                tcount = int(T[b].sum())
                nst = tcount // ST

                ps_qp = psD.tile([128, 128], dt.float32, tag="ps_epi")
                if has_bias:
                    nc.tensor.matmul(ps_qp[:], lhsT=ones[:], rhs=biases[:, 256:384],
                                     start=True, stop=False)
                    nc.tensor.matmul(ps_qp[:], lhsT=queryT[:, b * 128:(b + 1) * 128],
                                     rhs=wq[:], start=False, stop=True)
                else:
                    nc.tensor.matmul(ps_qp[:], lhsT=queryT[:, b * 128:(b + 1) * 128],
                                     rhs=wq[:], start=True, stop=True)
                qp_sb = epool.tile([128, 128], dt.bfloat16, tag="qp_sb")
                nc.scalar.activation(qp_sb[:], ps_qp[:], Act.Copy)

                kvbuf = bpool.tile([128, T_MAX, 256], dt.bfloat16, tag="kvbuf")
                if b < 2:  # first use of each pool slot: clear stale NaNs
                    nc.vector.memset(kvbuf[:], 0.0)
                for ch in range(nch):
                    tcc = int(T[b, ch])
                    base = ch * CHUNK
                    rows = min(CHUNK, nk_pad - base)
                    for g0 in range(0, tcc, GMAX):
                        gt = min(GMAX, tcc - g0)
                        toff = int(tile_base[b, ch]) - tb0 + g0
                        n_idx = gt * 128
                        i0 = ((int(tile_base[b, ch]) + g0) * 128) // 16
                        reg = cnt_regs[call_i % 4]
                        nc.gpsimd.reg_load(reg, gcnt[0:1, call_i:call_i + 1])
                        nc.gpsimd.dma_gather(
                            out_ap=kvbuf[:, toff:toff + gt, :],
                            in_ap=kv_dram[base:base + rows, :],
                            idxs_ap=kvidx[:, i0:i0 + n_idx // 16],
                            num_idxs=n_idx,
                            num_idxs_reg=reg,
                            elem_size=256,
                        )
                        call_i += 1

                qrow = bpool.tile([1, T_MAX * 128], dt.bfloat16, tag="qrow")
                nc.sync.dma_start(out=qrow[0:1, 0:tcount * 128],
                                  in_=p_qrelr[0:1, tb0 * 128:(tb0 + tcount) * 128])
                m_blk = bpool.tile([128, T_MAX * 128], dt.bfloat16, tag="m_blk")
                e_blk = bpool.tile([128, T_MAX * 8], dt.float32, tag="e_blk")
                p_blk = bpool.tile([128, T_MAX * 128], dt.bfloat16, tag="p_blk")

                # M for the whole block: M[e, n] = (q_rel[e] == n)
                qv = qrelp[:, tb0:tb0 + tcount].unsqueeze(-1).broadcast_to([128, tcount, 128])
                nc.vector.tensor_tensor(
                    m_blk[:, 0:tcount * 128].rearrange("p (t n) -> p t n", t=tcount),
                    iotar[:, 0:tcount * 128].rearrange("p (t n) -> p t n", t=tcount),
                    qv, op=Alu.is_equal)

                # pass 1: s = Qp[q_rel] + Kp, p = prelu(s)
                for st in range(nst):
                    t0 = st * ST
                    ps_b = psA.tile([128, 128 * ST], dt.float32, tag="ps_kv")
                    nc.tensor.matmul(ps_b[:], lhsT=ones[:],
                                     rhs=qrow[0:1, t0 * 128:(t0 + ST) * 128],
                                     start=True, stop=True)
                    b_sb = spool.tile([128, 128 * ST], dt.bfloat16, tag="b_sb")
                    nc.scalar.activation(b_sb[:], ps_b[:], Act.Copy)
                    mt_sb = spool.tile([128, 128 * ST], dt.bfloat16, tag="mt_sb")
                    nc.vector.tensor_scalar(mt_sb[:], b_sb[:], iotac[:], None,
                                            op0=Alu.is_equal)
                    ps_s = psB.tile([128, 128 * ST], dt.float32, tag="ps_s")
                    for t in range(ST):
                        nc.tensor.matmul(ps_s[:, t * 128:(t + 1) * 128],
                                         lhsT=mt_sb[:, t * 128:(t + 1) * 128],
                                         rhs=qp_sb[:], start=(t == 0), stop=False)
                    for t in range(ST):
                        nc.tensor.matmul(ps_s[:, t * 128:(t + 1) * 128],
                                         lhsT=ident[:],
                                         rhs=kvbuf[:, t0 + t, 0:128],
                                         start=False, stop=(t == ST - 1))
                    pv = p_blk[:, t0 * 128:(t0 + ST) * 128]
                    if USE_ACT_PRELU:
                        nc.scalar.activation(pv, ps_s[:], Act.Prelu, alpha=0.25)
                    else:
                        r_sb = spool.tile([128, 128 * ST], dt.bfloat16, tag="r_sb")
                        nc.scalar.activation(r_sb[:], ps_s[:], Act.Relu, scale=0.75)
                        nc.vector.scalar_tensor_tensor(pv, ps_s[:], 0.25, r_sb[:],
                                                       op0=Alu.mult, op1=Alu.add)

                # block-wide: pa = p * a, e = per-head sums, w = exp(e)
                pa_blk = bpool.tile([128, T_MAX * 128], dt.bfloat16, tag="pa_blk")
                nc.vector.tensor_tensor(pa_blk[:, 0:tcount * 128],
                                        p_blk[:, 0:tcount * 128],
                                        abc[:, 0:tcount * 128], op=Alu.mult)
                nc.vector.tensor_reduce(
                    e_blk[:, 0:tcount * 8].rearrange("p (t h) -> p t h", t=tcount),
                    pa_blk[:, 0:tcount * 128].rearrange("p (t h d) -> p t h d", t=tcount, h=H),
                    axis=mybir.AxisListType.X, op=Alu.add)
                w_blk = bpool.tile([128, T_MAX * 8], dt.bfloat16, tag="w_blk")
                nc.scalar.activation(w_blk[:, 0:tcount * 8], e_blk[:, 0:tcount * 8], Act.Exp)

                # block-wide: C = w * Vv
                c_blk = bpool.tile([128, T_MAX, 128], dt.bfloat16, tag="c_blk")
                wv_b = w_blk[:, 0:tcount * 8].rearrange("p (t h) -> p t h", t=tcount)
                wv_b = wv_b.unsqueeze(-1).broadcast_to([128, tcount, H, DH])
                nc.vector.tensor_tensor(
                    c_blk[:, 0:tcount, :].rearrange("p t (h d) -> p t h d", h=H),
                    wv_b,
                    kvbuf[:, 0:tcount, 128:256].rearrange("p t (h d) -> p t h d", h=H),
                    op=Alu.mult)

                # pass 2: scatter into PSUM accumulator via indicator matmuls
                ps_acc = psC.tile([128, 512], dt.float32, tag="ps_acc")
                for tt in range(tcount):
                    nc.tensor.matmul(ps_acc[:, 0:128],
                                     lhsT=m_blk[:, tt * 128:(tt + 1) * 128],
                                     rhs=c_blk[:, tt, :], start=(tt == 0), stop=False)
                    nc.tensor.matmul(ps_acc[:, 128:136],
                                     lhsT=m_blk[:, tt * 128:(tt + 1) * 128],
                                     rhs=w_blk[:, tt * 8:(tt + 1) * 8],
                                     start=False, stop=(tt == tcount - 1))

                # ---- epilogue ----
                lden = epool.tile([128, 8], dt.float32, tag="lden")
                nc.scalar.activation(lden[:], ps_acc[:, 128:136], Act.Ln, bias=eps30[:])
                recip = epool.tile([128, 8], dt.float32, tag="recip")
                nc.scalar.activation(recip[:], lden[:], Act.Exp, scale=-1.0)
                msgd = epool.tile([128, 128], dt.bfloat16, tag="msgd")
                rv = recip[:].unsqueeze(-1).broadcast_to([128, 8, DH])
                nc.vector.tensor_tensor(
                    msgd[:].rearrange("p (h d) -> p h d", h=H),
                    ps_acc[:, 0:128].rearrange("p (h d) -> p h d", h=H),
                    rv, op=Alu.mult)
                ps_t = psD.tile([128, 128], dt.bfloat16, tag="ps_epi")
                nc.tensor.transpose(ps_t[:], msgd[:], ident[:])
                mdT = epool.tile([128, 128], dt.bfloat16, tag="mdT")
                nc.scalar.activation(mdT[:], ps_t[:], Act.Copy)
                ps_o = psD.tile([128, 128], dt.float32, tag="ps_epi")
                if has_bias:
                    nc.tensor.matmul(ps_o[:], lhsT=ones[:], rhs=biases[:, 384:512],
                                     start=True, stop=False)
                    nc.tensor.matmul(ps_o[:], lhsT=mdT[:], rhs=wp[:], start=False, stop=True)
                else:
                    nc.tensor.matmul(ps_o[:], lhsT=mdT[:], rhs=wp[:], start=True, stop=True)
                qblk = epool.tile([128, 128], dt.float32, tag="qblk")
                nc.sync.dma_start(out=qblk[:], in_=p_query[b * 128:(b + 1) * 128, :])
                x_sb = epool.tile([128, 128], dt.float32, tag="x_sb")
                nc.vector.tensor_tensor(x_sb[:], ps_o[:], qblk[:], op=Alu.add)
                mu = epool.tile([128, 1], dt.float32, tag="mu")
                nc.vector.tensor_reduce(mu[:], x_sb[:], axis=mybir.AxisListType.X,
                                        op=Alu.add)
                mu_m = epool.tile([128, 1], dt.float32, tag="mu_m")
                nc.scalar.activation(mu_m[:], mu[:], Act.Copy, scale=1.0 / 128.0)
                xc = epool.tile([128, 128], dt.float32, tag="xc")
                nc.vector.tensor_scalar(xc[:], x_sb[:], mu_m[:], None, op0=Alu.subtract)
                sq = epool.tile([128, 128], dt.float32, tag="sq")
                nc.scalar.activation(sq[:], xc[:], Act.Square)
                var = epool.tile([128, 1], dt.float32, tag="var")
                nc.vector.tensor_reduce(var[:], sq[:], axis=mybir.AxisListType.X,
                                        op=Alu.add)
                lnv = epool.tile([128, 1], dt.float32, tag="lnv")
                nc.scalar.activation(lnv[:], var[:], Act.Ln, scale=1.0 / 128.0,
                                     bias=eps5[:])
                rstd = epool.tile([128, 1], dt.float32, tag="rstd")
                nc.scalar.activation(rstd[:], lnv[:], Act.Exp, scale=-0.5)
                y = epool.tile([128, 128], dt.float32, tag="y")
                nc.vector.tensor_scalar(y[:], xc[:], rstd[:], None, op0=Alu.mult)
                if has_bias:
                    yg = epool.tile([128, 128], dt.float32, tag="yg")
                    gb = lngb[:, 0:128].broadcast_to([128, 128])
                    nc.vector.tensor_tensor(yg[:], y[:], gb, op=Alu.mult)
                    bb = lngb[:, 128:256].broadcast_to([128, 128])
                    nc.vector.tensor_tensor(y[:], yg[:], bb, op=Alu.add)
                nc.sync.dma_start(out=p_out[b * 128:(b + 1) * 128, :], in_=y[:])

    nc.compile()
    return nc


# ----------------------------------------------------------------------------
# Public entry point
# ----------------------------------------------------------------------------


def kernel(query, keys, values, query_idx, key_idx, Wq, bq, Wk, bk, Wv, bv,
           Wp, bp, a, prelu_w, ln_g, ln_b, _want_trace=False):
    from concourse.bass_utils import run_bass_kernel_spmd

    query = np.asarray(query, np.float32)
    keys = np.asarray(keys, np.float32)
    values = np.asarray(values, np.float32)
    nq, dim = query.shape
    nk = keys.shape[0]
    assert dim == DIM
    pw = float(np.asarray(prelu_w).reshape(-1)[0])
    assert abs(pw - 0.25) < 1e-6, "prelu slope 0.25 is compiled in"

    has_bias = not (
        np.all(np.asarray(bq) == 0) and np.all(np.asarray(bk) == 0)
        and np.all(np.asarray(bv) == 0) and np.all(np.asarray(bp) == 0)
        and np.all(np.asarray(ln_g) == 1) and np.all(np.asarray(ln_b) == 0)
    )

    sched = _prep(query_idx, key_idx, nq, nk)
    npc, nblk = sched["npc"], sched["nblk"]
    T_MAX = sched["T_MAX"]
    npc_pad = nblk * 128
    nk_pad = ((nk + 127) // 128) * 128

    key_sched = (nq, nk, sched["S"], has_bias, sched["T"].tobytes())
    if key_sched not in _CACHE:
        _CACHE[key_sched] = _build(sched, nq, nk, has_bias)
    nc = _CACHE[key_sched]

    keysT = np.zeros((128, nk_pad), BF16)
    keysT[:, :nk] = keys.T.astype(BF16)
    valuesT = np.zeros((128, nk_pad), BF16)
    valuesT[:, :nk] = values.T.astype(BF16)
    wqT = np.ascontiguousarray(np.asarray(Wq, np.float32).T).astype(BF16)
    wkT = np.ascontiguousarray(np.asarray(Wk, np.float32).T).astype(BF16)
    wvT = np.ascontiguousarray(np.asarray(Wv, np.float32).T).astype(BF16)
    wpT = np.ascontiguousarray(np.asarray(Wp, np.float32).T).astype(BF16)
    a_flat = np.asarray(a, np.float32).reshape(-1)
    abc = np.tile(a_flat, (128, T_MAX)).astype(BF16)
    iotar = np.tile(np.arange(128, dtype=np.float32), (128, T_MAX)).astype(BF16)
    iotac = np.arange(128, dtype=np.float32).reshape(128, 1)
    ident = np.eye(128, dtype=np.float32).astype(BF16)
    ones = np.ones((1, 128), BF16)
    biases = np.concatenate([
        np.asarray(bk, np.float32), np.asarray(bv, np.float32),
        np.asarray(bq, np.float32), np.asarray(bp, np.float32)]).reshape(1, 512)
    lngb = np.concatenate([np.asarray(ln_g, np.float32),
                           np.asarray(ln_b, np.float32)]).reshape(1, 256)

    in_maps = []
    for c in range(N_CORES):
        qs = query[c * npc:(c + 1) * npc]
        qpadT = np.zeros((128, npc_pad), BF16)
        qpadT[:, :npc] = qs.T.astype(BF16)
        qpad = np.zeros((npc_pad, 128), np.float32)
        qpad[:npc] = qs
        m = {
            "kvidx": sched["kvidx"][c],
            "qrelp": sched["qrelp"][c],
            "qrelr": sched["qrelr"][c],
            "gcnt": sched["gcnt"][c].reshape(1, -1),
            "queryT": qpadT,
            "query": qpad,
            "keysT": keysT,
            "valuesT": valuesT,
            "wq": wqT, "wk": wkT, "wv": wvT, "wp": wpT,
            "abc": abc, "iotar": iotar, "iotac": iotac,
            "ident": ident, "ones": ones,
        }
        if has_bias:
            m["biases"] = biases
            m["lngb"] = lngb
        in_maps.append(m)

    res = run_bass_kernel_spmd(nc, in_maps, core_ids=list(range(N_CORES)),
                               trace=_want_trace)
    out = np.empty((nq, DIM), np.float32)
    for c in range(N_CORES):
        out[c * npc:(c + 1) * npc] = res.results[c]["out"][:npc]
    if _want_trace:
        kernel.last_exec_time_ns = res.exec_time_ns
        kernel.last_profile = res.profile_json
    return out


# revision 14
# speedup vs baseline: 1.0028x; 1.0028x over previous
"""Edge-parallel GNN message-passing layer on 8 TRN2 NeuronCores.

Sharding: each core owns NQ/8 query nodes and all edges pointing at them
(host pre-sorts edges by destination block); node features/weights are
replicated, so no collectives are needed — each core produces its output
rows independently.

Math notes: the reference's segment-max subtraction cancels exactly in
msg/denom and scores are small (|e| < ~5), so exp() is computed directly;
1/x and 1/sqrt(x) are computed as exp(-ln(x)) / exp(-0.5 ln(x)) so the
whole kernel stays in one ACT table set (natural_log_exp_and_others).
"""

import numpy as np
import ml_dtypes

BF16 = ml_dtypes.bfloat16

N_CORES = 8
DIM = 128
H = 8
DH = 16
CHUNK = 32768  # dma_gather int16 index reach (rows per gather table window)
ST = 4   # tiles per super-tile (512 edge slots)
GMAX = 8  # tiles per gather call (SWDGE descriptor-ring limit ~1024)
LN_EPS = 1e-5

USE_ACT_PRELU = True  # HW table has Prelu; interp does not — disable for sim

_CACHE = {}


def _subcalls(tcc):
    """Gather sub-call tile counts for a (block, chunk) with tcc tiles."""
    return [min(GMAX, tcc - g0) for g0 in range(0, tcc, GMAX)]


# ----------------------------------------------------------------------------
# Host-side prep
# ----------------------------------------------------------------------------


def _prep(query_idx, key_idx, nq, nk):
    npc = nq // N_CORES
    nblk = (npc + 127) // 128
    nch = (nk + CHUNK - 1) // CHUNK

    qi = np.asarray(query_idx).astype(np.int64)
    ki = np.asarray(key_idx).astype(np.int64)
    core = qi // npc

    per_core = []
    counts_all = np.zeros((N_CORES, nblk, nch), np.int64)
    for c in range(N_CORES):
        sel = np.nonzero(core == c)[0]
        eq = qi[sel] - c * npc
        ek = ki[sel]
        grp = (eq >> 7) * nch + (ek >> 15)
        order = np.argsort(grp, kind="stable")
        per_core.append((grp[order], (eq & 127)[order], (ek & (CHUNK - 1))[order]))
        counts_all[c] = np.bincount(grp, minlength=nblk * nch).reshape(nblk, nch)

    # Static tile schedule: T tiles per (block, chunk); gather calls cover
    # only the valid region R16 = round16(max-core count (+small filler)).
    maxc = counts_all.max(axis=0)  # [nblk, nch]
    R16 = ((maxc + 15) // 16) * 16
    R16 = np.maximum(R16, 16)
    T = (R16 + 127) // 128
    tot = T.sum(axis=1)
    T[:, nch - 1] += (-tot) % ST
    flat = T.reshape(-1)
    tb = np.concatenate([[0], np.cumsum(flat)[:-1]]).reshape(nblk, nch)
    total_tiles = int(flat.sum())
    S = total_tiles * 128

    kvidx = np.zeros((N_CORES, S), np.int16)
    qrel = np.full((N_CORES, S), -1.0, np.float32)

    grp_start_slot = tb.reshape(-1) * 128
    for c in range(N_CORES):
        grp_s, qrel_s, loc_s = per_core[c]
        cnt = counts_all[c].reshape(-1)
        starts = np.concatenate([[0], np.cumsum(cnt)[:-1]])
        rank = np.arange(grp_s.shape[0]) - np.repeat(starts, cnt)
        slot = grp_start_slot[grp_s] + rank
        kvidx[c, slot] = loc_s.astype(np.int16)
        qrel[c, slot] = qrel_s.astype(np.float32)
    ncalls = int(((R16 + 1023) // 1024).sum())

    kvidx_w = np.ascontiguousarray(kvidx.reshape(N_CORES, -1, 16).transpose(0, 2, 1))
    kvidx_w = np.ascontiguousarray(np.tile(kvidx_w, (1, 8, 1)))
    qrelp = np.ascontiguousarray(
        qrel.reshape(N_CORES, -1, 128).transpose(0, 2, 1)).astype(BF16)
    qrelr = qrel.reshape(N_CORES, 1, S).astype(BF16)

    return {
        "npc": npc, "nblk": nblk, "nch": nch, "T": T, "tile_base": tb,
        "R16": R16,
        "total_tiles": total_tiles, "S": S, "ncalls": ncalls,
        "T_MAX": int(T.sum(axis=1).max()),
        "kvidx": kvidx_w, "qrelp": qrelp, "qrelr": qrelr,
    }


# ----------------------------------------------------------------------------
# Device kernel
# ----------------------------------------------------------------------------


def _build(sched, nq, nk, has_bias):
    import concourse.bacc as bacc
    import concourse.mybir as mybir
    import concourse.tile as tile

    dt = mybir.dt
    Alu = mybir.AluOpType
    Act = mybir.ActivationFunctionType

    nblk, nch = sched["nblk"], sched["nch"]
    T, tile_base = sched["T"], sched["tile_base"]
    R16 = sched["R16"]
    S, ncalls, T_MAX = sched["S"], sched["ncalls"], sched["T_MAX"]
    npc_pad = nblk * 128
    nk_pad = ((nk + 127) // 128) * 128
    nkc = nk_pad // 128

    nc = bacc.Bacc(None)

    p_kvidx = nc.declare_dram_parameter("kvidx", [128, S // 16], dt.int16, isOutput=False)
    p_qrelp = nc.declare_dram_parameter("qrelp", [128, S // 128], dt.bfloat16, isOutput=False)
    p_qrelr = nc.declare_dram_parameter("qrelr", [1, S], dt.bfloat16, isOutput=False)
    p_queryT = nc.declare_dram_parameter("queryT", [128, npc_pad], dt.bfloat16, isOutput=False)
    p_query = nc.declare_dram_parameter("query", [npc_pad, 128], dt.float32, isOutput=False)
    p_keysT = nc.declare_dram_parameter("keysT", [128, nk_pad], dt.bfloat16, isOutput=False)
    p_valuesT = nc.declare_dram_parameter("valuesT", [128, nk_pad], dt.bfloat16, isOutput=False)
    p_wq = nc.declare_dram_parameter("wq", [128, 128], dt.bfloat16, isOutput=False)
    p_wk = nc.declare_dram_parameter("wk", [128, 128], dt.bfloat16, isOutput=False)
    p_wv = nc.declare_dram_parameter("wv", [128, 128], dt.bfloat16, isOutput=False)
    p_wp = nc.declare_dram_parameter("wp", [128, 128], dt.bfloat16, isOutput=False)
    p_abc = nc.declare_dram_parameter("abc", [128, 128 * T_MAX], dt.bfloat16, isOutput=False)
    p_iotar = nc.declare_dram_parameter("iotar", [128, 128 * T_MAX], dt.bfloat16, isOutput=False)
    p_iotac = nc.declare_dram_parameter("iotac", [128, 1], dt.float32, isOutput=False)
    p_ident = nc.declare_dram_parameter("ident", [128, 128], dt.bfloat16, isOutput=False)
    p_ones = nc.declare_dram_parameter("ones", [1, 128], dt.bfloat16, isOutput=False)
    if has_bias:
        p_biases = nc.declare_dram_parameter("biases", [1, 512], dt.float32, isOutput=False)
        p_lngb = nc.declare_dram_parameter("lngb", [1, 256], dt.float32, isOutput=False)
    p_out = nc.declare_dram_parameter("out", [npc_pad, 128], dt.float32, isOutput=True)

    kv_dram = nc.dram_tensor("kv_table", [nk_pad, 256], dt.bfloat16)

    KVG = 8

    with tile.TileContext(nc) as tc:
        with (
            tc.tile_pool(name="const", bufs=1) as cpool,
            tc.tile_pool(name="kstream", bufs=3) as kpool,
            tc.tile_pool(name="kvout", bufs=3) as kvopool,
            tc.tile_pool(name="blk", bufs=2) as bpool,
            tc.tile_pool(name="stile", bufs=3) as spool,
            tc.tile_pool(name="epi", bufs=2) as epool,
            tc.tile_pool(name="psA", bufs=2, space="PSUM") as psA,
            tc.tile_pool(name="psB", bufs=2, space="PSUM") as psB,
            tc.tile_pool(name="psC", bufs=2, space="PSUM") as psC,
            tc.tile_pool(name="psD", bufs=2, space="PSUM") as psD,
        ):
            def cload(param, shape, dtype):
                t = cpool.tile(shape, dtype, tag=param.name)
                nc.sync.dma_start(out=t[:], in_=param[:])
                return t

            eps30 = cpool.tile([128, 1], dt.float32, tag="eps30")
            nc.gpsimd.memset(eps30[:], 1e-30)
            eps5 = cpool.tile([128, 1], dt.float32, tag="eps5")
            nc.gpsimd.memset(eps5[:], LN_EPS)
            wq = cload(p_wq, [128, 128], dt.bfloat16)
            wk = cload(p_wk, [128, 128], dt.bfloat16)
            wv = cload(p_wv, [128, 128], dt.bfloat16)
            wp = cload(p_wp, [128, 128], dt.bfloat16)
            abc = cload(p_abc, [128, 128 * T_MAX], dt.bfloat16)
            iotar = cload(p_iotar, [128, 128 * T_MAX], dt.bfloat16)
            iotac = cload(p_iotac, [128, 1], dt.float32)
            ident = cload(p_ident, [128, 128], dt.bfloat16)
            ones = cload(p_ones, [1, 128], dt.bfloat16)
            kvidx = cload(p_kvidx, [128, S // 16], dt.int16)
            qrelp = cload(p_qrelp, [128, S // 128], dt.bfloat16)
            queryT = cload(p_queryT, [128, npc_pad], dt.bfloat16)
            if has_bias:
                biases = cload(p_biases, [1, 512], dt.float32)
                lngb = cload(p_lngb, [1, 256], dt.float32)

            # ---- Phase A: KV table = [keys@Wk.T | values@Wv.T] bf16 ----
            for jg0 in range(0, nkc, KVG):
                gsz = min(KVG, nkc - jg0)
                kT = kpool.tile([128, 128 * KVG], dt.bfloat16, tag="kT")
                vT = kpool.tile([128, 128 * KVG], dt.bfloat16, tag="vT")
                nc.sync.dma_start(out=kT[:, 0:128 * gsz], in_=p_keysT[:, jg0 * 128:(jg0 + gsz) * 128])
                nc.sync.dma_start(out=vT[:, 0:128 * gsz], in_=p_valuesT[:, jg0 * 128:(jg0 + gsz) * 128])
                kv_sb = kvopool.tile([128, KVG, 256], dt.bfloat16, tag="kv_sb")
                for j in range(gsz):
                    ps = psA.tile([128, 512], dt.float32, tag="ps_kv")
                    if has_bias:
                        nc.tensor.matmul(ps[:, 0:256], lhsT=ones[:], rhs=biases[:, 0:256],
                                         start=True, stop=False)
                        nc.tensor.matmul(ps[:, 0:128], lhsT=kT[:, j * 128:(j + 1) * 128],
                                         rhs=wk[:], start=False, stop=False)
                        nc.tensor.matmul(ps[:, 128:256], lhsT=vT[:, j * 128:(j + 1) * 128],
                                         rhs=wv[:], start=False, stop=True)
                    else:
                        nc.tensor.matmul(ps[:, 0:128], lhsT=kT[:, j * 128:(j + 1) * 128],
                                         rhs=wk[:], start=True, stop=False)
                        nc.tensor.matmul(ps[:, 128:256], lhsT=vT[:, j * 128:(j + 1) * 128],
                                         rhs=wv[:], start=False, stop=True)
                    if j % 2 == 0:
                        nc.scalar.activation(kv_sb[:, j, :], ps[:, 0:256], Act.Copy)
                    else:
                        nc.vector.tensor_copy(kv_sb[:, j, :], ps[:, 0:256])
                dview = kv_dram[jg0 * 128:(jg0 + gsz) * 128, :]
                dview = dview.rearrange("(j p) d -> p j d", p=128)
                nc.sync.dma_start(out=dview, in_=kv_sb[:, 0:gsz, :])

            # ---- Phase B ----
            call_i = 0
            for b in range(nblk):
                tb0 = int(tile_base[b, 0])
                tcount = int(T[b].sum())
                nst = tcount // ST

                ps_qp = psD.tile([128, 128], dt.float32, tag="ps_epi")
                if has_bias:
                    nc.tensor.matmul(ps_qp[:], lhsT=ones[:], rhs=biases[:, 256:384],
                                     start=True, stop=False)
                    nc.tensor.matmul(ps_qp[:], lhsT=queryT[:, b * 128:(b + 1) * 128],
                                     rhs=wq[:], start=False, stop=True)
                else:
                    nc.tensor.matmul(ps_qp[:], lhsT=queryT[:, b * 128:(b + 1) * 128],
                                     rhs=wq[:], start=True, stop=True)
                qp_sb = epool.tile([128, 128], dt.bfloat16, tag="qp_sb")
                nc.scalar.activation(qp_sb[:], ps_qp[:], Act.Copy)

                kvbuf = bpool.tile([128, T_MAX, 256], dt.bfloat16, tag="kvbuf")
                if b < 2:  # first use of each pool slot: clear stale NaNs
                    nc.vector.memset(kvbuf[:], 0.0)
                for ch in range(nch):
                    rr = int(R16[b, ch])
                    base = ch * CHUNK
                    rows = min(CHUNK, nk_pad - base)
                    slot0 = int(tile_base[b, ch]) * 128
                    for g0 in range(0, rr, 1024):
                        n_idx = min(1024, rr - g0)
                        toff = int(tile_base[b, ch]) - tb0 + g0 // 128
                        ntl = (n_idx + 127) // 128
                        i0 = (slot0 + g0) // 16
                        nc.gpsimd.dma_gather(
                            out_ap=kvbuf[:, toff:toff + ntl, :],
                            in_ap=kv_dram[base:base + rows, :],
                            idxs_ap=kvidx[:, i0:i0 + n_idx // 16],
                            num_idxs=n_idx,
                            num_idxs_reg=n_idx,
                            elem_size=256,
                        )
                        call_i += 1

                qrow = bpool.tile([1, T_MAX * 128], dt.bfloat16, tag="qrow")
                nc.sync.dma_start(out=qrow[0:1, 0:tcount * 128],
                                  in_=p_qrelr[0:1, tb0 * 128:(tb0 + tcount) * 128])
                m_blk = bpool.tile([128, T_MAX * 128], dt.bfloat16, tag="m_blk")
                e_blk = bpool.tile([128, T_MAX * 8], dt.float32, tag="e_blk")
                p_blk = bpool.tile([128, T_MAX * 128], dt.bfloat16, tag="p_blk")

                # M for the whole block: M[e, n] = (q_rel[e] == n)
                qv = qrelp[:, tb0:tb0 + tcount].unsqueeze(-1).broadcast_to([128, tcount, 128])
                nc.vector.tensor_tensor(
                    m_blk[:, 0:tcount * 128].rearrange("p (t n) -> p t n", t=tcount),
                    iotar[:, 0:tcount * 128].rearrange("p (t n) -> p t n", t=tcount),
                    qv, op=Alu.is_equal)

                # pass 1: s = Qp[q_rel] + Kp, p = prelu(s)
                for st in range(nst):
                    t0 = st * ST
                    ps_b = psA.tile([128, 128 * ST], dt.float32, tag="ps_kv")
                    nc.tensor.matmul(ps_b[:], lhsT=ones[:],
                                     rhs=qrow[0:1, t0 * 128:(t0 + ST) * 128],
                                     start=True, stop=True)
                    b_sb = spool.tile([128, 128 * ST], dt.bfloat16, tag="b_sb")
                    nc.scalar.activation(b_sb[:], ps_b[:], Act.Copy)
                    mt_sb = spool.tile([128, 128 * ST], dt.bfloat16, tag="mt_sb")
                    nc.vector.tensor_scalar(mt_sb[:], b_sb[:], iotac[:], None,
                                            op0=Alu.is_equal)
                    ps_s = psB.tile([128, 128 * ST], dt.float32, tag="ps_s")
                    for t in range(ST):
                        nc.tensor.matmul(ps_s[:, t * 128:(t + 1) * 128],
                                         lhsT=mt_sb[:, t * 128:(t + 1) * 128],
                                         rhs=qp_sb[:], start=(t == 0), stop=False)
                    for t in range(ST):
                        nc.tensor.matmul(ps_s[:, t * 128:(t + 1) * 128],
                                         lhsT=ident[:],
                                         rhs=kvbuf[:, t0 + t, 0:128],
                                         start=False, stop=(t == ST - 1))
                    pv = p_blk[:, t0 * 128:(t0 + ST) * 128]
                    if USE_ACT_PRELU:
                        nc.scalar.activation(pv, ps_s[:], Act.Prelu, alpha=0.25)
                    else:
                        r_sb = spool.tile([128, 128 * ST], dt.bfloat16, tag="r_sb")
                        nc.scalar.activation(r_sb[:], ps_s[:], Act.Relu, scale=0.75)
                        nc.vector.scalar_tensor_tensor(pv, ps_s[:], 0.25, r_sb[:],
                                                       op0=Alu.mult, op1=Alu.add)

                # block-wide: pa = p * a, e = per-head sums, w = exp(e)
                pa_blk = bpool.tile([128, T_MAX * 128], dt.bfloat16, tag="pa_blk")
                nc.vector.tensor_tensor(pa_blk[:, 0:tcount * 128],
                                        p_blk[:, 0:tcount * 128],
                                        abc[:, 0:tcount * 128], op=Alu.mult)
                nc.vector.tensor_reduce(
                    e_blk[:, 0:tcount * 8].rearrange("p (t h) -> p t h", t=tcount),
                    pa_blk[:, 0:tcount * 128].rearrange("p (t h d) -> p t h d", t=tcount, h=H),
                    axis=mybir.AxisListType.X, op=Alu.add)
                # w expanded to [t, h, dh] on ACT (reads e with a stride-0 AP)
                w_exp = bpool.tile([128, T_MAX, H, DH], dt.bfloat16, tag="w_exp")
                ev = e_blk[:, 0:tcount * 8].rearrange("p (t h) -> p t h", t=tcount)
                ev = ev.unsqueeze(-1).broadcast_to([128, tcount, H, DH])
                nc.scalar.activation(w_exp[:, 0:tcount], ev, Act.Exp)

                # block-wide: C = w * Vv (both contiguous bf16 -> DVE 2x mode)
                c_blk = bpool.tile([128, T_MAX, 128], dt.bfloat16, tag="c_blk")
                nc.vector.tensor_tensor(
                    c_blk[:, 0:tcount, :],
                    w_exp[:, 0:tcount].rearrange("p t h d -> p t (h d)"),
                    kvbuf[:, 0:tcount, 128:256],
                    op=Alu.mult)

                # pass 2: scatter into PSUM accumulator via indicator matmuls
                ps_acc = psC.tile([128, 512], dt.float32, tag="ps_acc")
                for tt in range(tcount):
                    nc.tensor.matmul(ps_acc[:, 0:128],
                                     lhsT=m_blk[:, tt * 128:(tt + 1) * 128],
                                     rhs=c_blk[:, tt, :], start=(tt == 0), stop=False)
                    nc.tensor.matmul(ps_acc[:, 128:136],
                                     lhsT=m_blk[:, tt * 128:(tt + 1) * 128],
                                     rhs=w_exp[:, tt, :, 0],
                                     start=False, stop=(tt == tcount - 1))

                # ---- epilogue ----
                lden = epool.tile([128, 8], dt.float32, tag="lden")
                nc.scalar.activation(lden[:], ps_acc[:, 128:136], Act.Ln, bias=eps30[:])
                recip = epool.tile([128, 8], dt.float32, tag="recip")
                nc.scalar.activation(recip[:], lden[:], Act.Exp, scale=-1.0)
                msgd = epool.tile([128, 128], dt.bfloat16, tag="msgd")
                rv = recip[:].unsqueeze(-1).broadcast_to([128, 8, DH])
                nc.vector.tensor_tensor(
                    msgd[:].rearrange("p (h d) -> p h d", h=H),
                    ps_acc[:, 0:128].rearrange("p (h d) -> p h d", h=H),
                    rv, op=Alu.mult)
                ps_t = psD.tile([128, 128], dt.bfloat16, tag="ps_epi")
                nc.tensor.transpose(ps_t[:], msgd[:], ident[:])
                mdT = epool.tile([128, 128], dt.bfloat16, tag="mdT")
                nc.scalar.activation(mdT[:], ps_t[:], Act.Copy)
                ps_o = psD.tile([128, 128], dt.float32, tag="ps_epi")
                if has_bias:
                    nc.tensor.matmul(ps_o[:], lhsT=ones[:], rhs=biases[:, 384:512],
                                     start=True, stop=False)
                    nc.tensor.matmul(ps_o[:], lhsT=mdT[:], rhs=wp[:], start=False, stop=True)
                else:
                    nc.tensor.matmul(ps_o[:], lhsT=mdT[:], rhs=wp[:], start=True, stop=True)
                qblk = epool.tile([128, 128], dt.float32, tag="qblk")
                nc.sync.dma_start(out=qblk[:], in_=p_query[b * 128:(b + 1) * 128, :])
                x_sb = epool.tile([128, 128], dt.float32, tag="x_sb")
                nc.vector.tensor_tensor(x_sb[:], ps_o[:], qblk[:], op=Alu.add)
                mu = epool.tile([128, 1], dt.float32, tag="mu")
                nc.vector.tensor_reduce(mu[:], x_sb[:], axis=mybir.AxisListType.X,
                                        op=Alu.add)
                mu_m = epool.tile([128, 1], dt.float32, tag="mu_m")
                nc.scalar.activation(mu_m[:], mu[:], Act.Copy, scale=1.0 / 128.0)
                xc = epool.tile([128, 128], dt.float32, tag="xc")
                nc.vector.tensor_scalar(xc[:], x_sb[:], mu_m[:], None, op0=Alu.subtract)
                sq = epool.tile([128, 128], dt.float32, tag="sq")
                nc.scalar.activation(sq[:], xc[:], Act.Square)
                var = epool.tile([128, 1], dt.float32, tag="var")
                nc.vector.tensor_reduce(var[:], sq[:], axis=mybir.AxisListType.X,
                                        op=Alu.add)
                lnv = epool.tile([128, 1], dt.float32, tag="lnv")
                nc.scalar.activation(lnv[:], var[:], Act.Ln, scale=1.0 / 128.0,
                                     bias=eps5[:])
                rstd = epool.tile([128, 1], dt.float32, tag="rstd")
                nc.scalar.activation(rstd[:], lnv[:], Act.Exp, scale=-0.5)
                y = epool.tile([128, 128], dt.float32, tag="y")
                nc.vector.tensor_scalar(y[:], xc[:], rstd[:], None, op0=Alu.mult)
                if has_bias:
                    yg = epool.tile([128, 128], dt.float32, tag="yg")
                    gb = lngb[:, 0:128].broadcast_to([128, 128])
                    nc.vector.tensor_tensor(yg[:], y[:], gb, op=Alu.mult)
                    bb = lngb[:, 128:256].broadcast_to([128, 128])
                    nc.vector.tensor_tensor(y[:], yg[:], bb, op=Alu.add)
                nc.sync.dma_start(out=p_out[b * 128:(b + 1) * 128, :], in_=y[:])

    nc.compile()
    return nc


# ----------------------------------------------------------------------------
# Public entry point
# ----------------------------------------------------------------------------


def kernel(query, keys, values, query_idx, key_idx, Wq, bq, Wk, bk, Wv, bv,
           Wp, bp, a, prelu_w, ln_g, ln_b, _want_trace=False):
    from concourse.bass_utils import run_bass_kernel_spmd

    query = np.asarray(query, np.float32)
    keys = np.asarray(keys, np.float32)
    values = np.asarray(values, np.float32)
    nq, dim = query.shape
    nk = keys.shape[0]
    assert dim == DIM
    pw = float(np.asarray(prelu_w).reshape(-1)[0])
    assert abs(pw - 0.25) < 1e-6, "prelu slope 0.25 is compiled in"

    has_bias = not (
        np.all(np.asarray(bq) == 0) and np.all(np.asarray(bk) == 0)
        and np.all(np.asarray(bv) == 0) and np.all(np.asarray(bp) == 0)
        and np.all(np.asarray(ln_g) == 1) and np.all(np.asarray(ln_b) == 0)
    )

    sched = _prep(query_idx, key_idx, nq, nk)
    npc, nblk = sched["npc"], sched["nblk"]
    T_MAX = sched["T_MAX"]
    npc_pad = nblk * 128
    nk_pad = ((nk + 127) // 128) * 128

    key_sched = (nq, nk, sched["S"], has_bias, sched["T"].tobytes())
    if key_sched not in _CACHE:
        _CACHE[key_sched] = _build(sched, nq, nk, has_bias)
    nc = _CACHE[key_sched]

    keysT = np.zeros((128, nk_pad), BF16)
    keysT[:, :nk] = keys.T.astype(BF16)
    valuesT = np.zeros((128, nk_pad), BF16)
    valuesT[:, :nk] = values.T.astype(BF16)
    wqT = np.ascontiguousarray(np.asarray(Wq, np.float32).T).astype(BF16)
    wkT = np.ascontiguousarray(np.asarray(Wk, np.float32).T).astype(BF16)
    wvT = np.ascontiguousarray(np.asarray(Wv, np.float32).T).astype(BF16)
    wpT = np.ascontiguousarray(np.asarray(Wp, np.float32).T).astype(BF16)
    a_flat = np.asarray(a, np.float32).reshape(-1)
    abc = np.tile(a_flat, (128, T_MAX)).astype(BF16)
    iotar = np.tile(np.arange(128, dtype=np.float32), (128, T_MAX)).astype(BF16)
    iotac = np.arange(128, dtype=np.float32).reshape(128, 1)
    ident = np.eye(128, dtype=np.float32).astype(BF16)
    ones = np.ones((1, 128), BF16)
    biases = np.concatenate([
        np.asarray(bk, np.float32), np.asarray(bv, np.float32),
        np.asarray(bq, np.float32), np.asarray(bp, np.float32)]).reshape(1, 512)
    lngb = np.concatenate([np.asarray(ln_g, np.float32),
                           np.asarray(ln_b, np.float32)]).reshape(1, 256)

    in_maps = []
    for c in range(N_CORES):
        qs = query[c * npc:(c + 1) * npc]
        qpadT = np.zeros((128, npc_pad), BF16)
        qpadT[:, :npc] = qs.T.astype(BF16)
        qpad = np.zeros((npc_pad, 128), np.float32)
        qpad[:npc] = qs
        m = {
            "kvidx": sched["kvidx"][c],
            "qrelp": sched["qrelp"][c],
            "qrelr": sched["qrelr"][c],
            "queryT": qpadT,
            "query": qpad,
            "keysT": keysT,
            "valuesT": valuesT,
            "wq": wqT, "wk": wkT, "wv": wvT, "wp": wpT,
            "abc": abc, "iotar": iotar, "iotac": iotac,
            "ident": ident, "ones": ones,
        }
        if has_bias:
            m["biases"] = biases
            m["lngb"] = lngb
        in_maps.append(m)

    res = run_bass_kernel_spmd(nc, in_maps, core_ids=list(range(N_CORES)),
                               trace=_want_trace)
    out = np.empty((nq, DIM), np.float32)
    for c in range(N_CORES):
        out[c * npc:(c + 1) * npc] = res.results[c]["out"][:npc]
    if _want_trace:
        kernel.last_exec_time_ns = res.exec_time_ns
        kernel.last_profile = res.profile_json
    return out
